# revision 54
# baseline (speedup 1.0000x reference)
"""Trainium2 Bass kernel for GNN message-passing encoder.

Computes (reference semantics):
    node_h = relu(emb[node_tokens] @ w1 + b1)        [N, D]
    edge_h = relu(emb[edge_tokens] @ w2 + b2)        [E, D]
    msg    = node_h[src] * edge_h                    [E, D]
    out    = segment_sum(msg, dst, N)                [N, D]

Strategy (8 NeuronCores):
  * Algebraic rewrite: both MLPs act on embedding rows, so precompute
    transformed tables R1 = relu(emb@w1+b1), R2 = relu(emb@w2+b2)
    (VOCAB rows each) once, then the per-edge work is two row gathers
    (R1[node_tokens[src]], R2[edge_tokens]), an elementwise multiply and
    a segment-sum.  This removes the per-edge matmuls entirely.
  * Phase A: each core computes a 1/8 row-shard of (R1|R2) packed as
    [VPAD/8, 256] and an AllGather replicates the full [VPAD, 256] table.
  * Phase B: edges are sorted by dst and dst-range sharded across cores
    (6272 nodes = 49 blocks of 128 per core).  Per 128-node block the
    edges are gathered with dma_gather (int16 row indices, 512B rows),
    msg = u*v on DVE, and a one-hot matrix S (built on DVE from the
    per-edge local dst id vs an iota row) is used on the PE:
    PSUM[128 nodes, D] += S^T @ msg accumulated over the block's edge
    tiles -- a dense segment-sum with no scatter and no collective.
"""

import contextlib

import numpy as np

import ml_dtypes

import concourse.bacc as bacc
import concourse.bass as bass
import concourse.mybir as mybir
import concourse.tile as tile
from concourse import library_config
from concourse.bass_utils import run_bass_kernel_spmd

F32 = mybir.dt.float32
F16 = mybir.dt.float16
BF16 = mybir.dt.bfloat16
I16 = mybir.dt.int16
NP_BF16 = ml_dtypes.bfloat16
_noop_ctx = contextlib.nullcontext

C = 8          # cores
D = 128        # feature dim
P = 128        # partitions
RA = 24        # u-gather runahead blocks (split pipeline)
UBUFS = RA + 2


class Cfg:
    def __init__(self, n_nodes, n_edges, vocab, v_pad, blocks_pc):
        self.n_nodes = n_nodes
        self.n_edges = n_edges
        self.vocab = vocab
        self.v_pad = v_pad              # multiple of 8*128
        self.blocks_pc = blocks_pc      # node blocks (128 nodes) per core
        self.npc = blocks_pc * P        # nodes per core (padded)
        assert self.npc * C >= n_nodes
        assert v_pad % (C * P) == 0
        assert v_pad <= 32768           # int16 gather indices


FULL_CFG = Cfg(n_nodes=50000, n_edges=600000, vocab=32000, v_pad=32768,
               blocks_pc=49)


def host_prep(cfg, emb_table, w1, bias1, w2, bias2, node_tokens, edge_tokens,
              src, dst, neg_pads=False):
    """Pure index/layout prep on host -> per-core input maps + K_list."""
    emb_table = np.asarray(emb_table, np.float32)
    w1 = np.asarray(w1, NP_BF16)
    w2 = np.asarray(w2, NP_BF16)
    bias1 = np.asarray(bias1, NP_BF16).reshape(1, D)
    bias2 = np.asarray(bias2, NP_BF16).reshape(1, D)
    node_tokens = np.asarray(node_tokens).astype(np.int64)
    edge_tokens = np.asarray(edge_tokens).astype(np.int64)
    src = np.asarray(src).astype(np.int64)
    dst = np.asarray(dst).astype(np.int64)

    stok = node_tokens[src]                      # token feeding node_h per edge
    order = np.argsort(dst, kind="stable")
    dstS = dst[order]
    stokS = stok[order]
    etokS = edge_tokens[order]
    gdt_np = np.float16

    nblk = C * cfg.blocks_pc
    bounds = np.searchsorted(dstS, np.arange(nblk + 1) * P)
    cnt = np.diff(bounds).reshape(C, cfg.blocks_pc)   # [C, blocks_pc]
    # order-statistic rebalancing: slot j holds each core's j-th largest
    # block, so the SPMD max-over-cores padding shrinks from ~8% to ~2%.
    # perms[c, j] = original block index served in slot j on core c; the
    # caller un-permutes the per-slot output rows.
    perms = np.argsort(-cnt, axis=1, kind="stable")   # [C, blocks_pc]
    cnt_slot = np.take_along_axis(cnt, perms, axis=1)
    nmax = cnt_slot.max(axis=0)                       # per slot max count
    K_list = np.maximum(1, -(-nmax // P)).astype(int)  # tiles per slot
    toff = np.concatenate([[0], np.cumsum(K_list)]).astype(int)
    tiles_total = int(toff[-1])
    slots = tiles_total * P

    sh = cfg.v_pad // C

    iota = np.tile(np.arange(P, dtype=gdt_np)[None, :], (P, 1))
    ones = np.ones((1, D), NP_BF16)

    def pack16(a):
        # gather index packing: idx k lives at [k%16, k//16], replicated to
        # 128 partitions (8 gpsimd cores x 16)
        return np.ascontiguousarray(np.tile(a.reshape(-1, 16).T, (8, 1)))

    # pad slots gather real rows spread over the table (the one-hot S has
    # zero rows there, killing their contribution).  NOTE: trailing -1
    # indices (neg_pads=True), although documented as "ignored", wedge the
    # device on this runtime even with >=1 valid index per call -- do not
    # re-enable without re-testing.
    spread = (np.arange(slots, dtype=np.int64) * 97 % cfg.vocab).astype(
        np.int16)
    pad_fill = np.full(slots, -1, np.int16) if neg_pads else spread
    in_maps = []
    for c in range(C):
        u16 = pad_fill.copy()
        v16 = pad_fill.copy()
        col = np.full(slots, -1.0, gdt_np)
        for j in range(cfg.blocks_pc):
            b = int(perms[c, j])
            gb = c * cfg.blocks_pc + b
            s0, s1 = bounds[gb], bounds[gb + 1]
            m = s1 - s0
            o = toff[j] * P
            u16[o:o + m] = stokS[s0:s1].astype(np.int16)
            v16[o:o + m] = etokS[s0:s1].astype(np.int16)
            col[o:o + m] = (dstS[s0:s1] - (c * cfg.npc + b * P)).astype(
                gdt_np)

        if neg_pads:
            # a fully-negative gather call (0 descriptors) wedges the
            # device; keep >=1 valid index at each call's start
            for b in range(cfg.blocks_pc):
                K = int(K_list[b])
                ni = K * P
                m = int(cnt[c, b])
                nch = -(-ni // 1024)            # GMAX
                csz = (-(-K // nch)) * P
                o = toff[b] * P
                for c0 in range(0, ni, csz):
                    if c0 >= m:
                        u16[o + c0] = 0
                        v16[o + c0] = 0

        # per-gather-call true edge counts (shared by the u and v call of
        # the same chunk): clamp to >=16 so no call is empty, and to the
        # chunk size.  Slots beyond the count keep stale-but-finite data
        # (primed buffers) and are killed by the zero rows of S.
        gcnt = []
        for j in range(cfg.blocks_pc):
            K = int(K_list[j])
            ni = K * P
            m = int(cnt_slot[c, j])
            nch = -(-ni // 1024)                # GMAX
            csz = (-(-K // nch)) * P
            for c0 in range(0, ni, csz):
                gcnt.append(int(np.clip(m - c0, 16, min(csz, ni - c0))))
        gcnt = np.asarray(gcnt, np.int32).reshape(1, -1)

        shard = np.zeros((sh, D), np.float32)
        lo, hi = c * sh, min((c + 1) * sh, cfg.vocab)
        if lo < cfg.vocab:
            shard[:hi - lo] = emb_table[lo:hi]

        in_maps.append({
            "emb_shT": np.ascontiguousarray(shard.T).astype(
                NP_BF16),                                   # [128, sh]
            "w1": w1, "w2": w2, "b1": bias1, "b2": bias2,
            "ones": ones, "iota": iota,
            "idx_u": pack16(u16),                           # [128, tiles*8]
            "idx_v": pack16(v16),
            "gcnt": gcnt,                                   # [1, ncalls]
            "col": np.ascontiguousarray(
                col.reshape(tiles_total, P).T),             # [128, tiles]
        })
    return in_maps, K_list, perms


def _call_offsets(cfg, K_list):
    """Per-block offsets into the per-call gcnt array (chunk structure must
    match _chunks)."""
    coff = []
    n = 0
    for b in range(cfg.blocks_pc):
        coff.append(n)
        K = int(K_list[b])
        ni = K * P
        nch = -(-ni // 1024)
        csz = (-(-K // nch)) * P
        n += len(range(0, ni, csz))
    return coff, n


def build_nc(cfg, K_list, repeat_b=1, repeat_a=1, repeat_all=1,
             table_dtype=F16, prime=True, repeat_ag=1, split_ag=True,
             rt_counts=False):
    # NOTE rt_counts (runtime per-call gather counts via gpsimd reg_load)
    # hangs the device: the tile scheduler does not order reg_load vs the
    # consuming dma_gather, so counts race.  Left off.
    """repeat_* repeat phase bodies / the whole pipeline inside one NEFF;
    only used by the timing harness to amortize launch overhead (results
    unchanged)."""
    GDT = table_dtype
    sh = cfg.v_pad // C
    st = sh // P                      # shard tiles (phase A)
    toff = np.concatenate([[0], np.cumsum(K_list)]).astype(int)
    tiles_total = int(toff[-1])
    coff, ncalls = _call_offsets(cfg, K_list)

    nc = bacc.Bacc("TRN2", target_bir_lowering=False, num_devices=C,
                   num_swdge_queues=4)

    p_embT = nc.declare_dram_parameter("emb_shT", [P, sh], BF16,
                                       isOutput=False)
    p_w1 = nc.declare_dram_parameter("w1", [D, D], BF16, isOutput=False)
    p_w2 = nc.declare_dram_parameter("w2", [D, D], BF16, isOutput=False)
    p_b1 = nc.declare_dram_parameter("b1", [1, D], BF16, isOutput=False)
    p_b2 = nc.declare_dram_parameter("b2", [1, D], BF16, isOutput=False)
    p_ones = nc.declare_dram_parameter("ones", [1, D], BF16, isOutput=False)
    p_iota = nc.declare_dram_parameter("iota", [P, P], GDT, isOutput=False)
    p_idxu = nc.declare_dram_parameter("idx_u", [P, tiles_total * 8], I16,
                                       isOutput=False)
    p_idxv = nc.declare_dram_parameter("idx_v", [P, tiles_total * 8], I16,
                                       isOutput=False)
    p_col = nc.declare_dram_parameter("col", [P, tiles_total], GDT,
                                      isOutput=False)
    p_gcnt = nc.declare_dram_parameter("gcnt", [1, ncalls], mybir.dt.int32,
                                       isOutput=False)
    p_out = nc.declare_dram_parameter("out", [cfg.npc, D], GDT,
                                      isOutput=True)

    with tile.TileContext(nc) as tc:
        with (
            tc.tile_pool(name="dram", bufs=1, space="DRAM") as dramp,
            tc.tile_pool(name="cst", bufs=1) as cst,
            tc.tile_pool(name="pa", bufs=3) as pa,
            tc.tile_pool(name="psA", bufs=2, space="PSUM") as psA,
            tc.tile_pool(name="gat", bufs=4) as gat,
            tc.tile_pool(name="sm", bufs=6) as smp,
            tc.tile_pool(name="psB", bufs=4, space="PSUM") as psB,
            tc.tile_pool(name="fl", bufs=3) as flp,
        ):
            w1_sb = cst.tile([D, D], BF16)
            nc.sync.dma_start(w1_sb[:], p_w1[:])
            w2_sb = cst.tile([D, D], BF16)
            nc.sync.dma_start(w2_sb[:], p_w2[:])
            b1_sb = cst.tile([1, D], BF16)
            nc.sync.dma_start(b1_sb[:], p_b1[:])
            b2_sb = cst.tile([1, D], BF16)
            nc.sync.dma_start(b2_sb[:], p_b2[:])
            ones_sb = cst.tile([1, D], BF16)
            nc.sync.dma_start(ones_sb[:], p_ones[:])
            iota_sb = cst.tile([P, P], GDT)
            nc.sync.dma_start(iota_sb[:], p_iota[:])
            embT_sb = cst.tile([P, sh], BF16)
            nc.sync.dma_start(embT_sb[:], p_embT[:])
            idxu_sb = cst.tile([P, tiles_total * 8], I16)
            nc.sync.dma_start(idxu_sb[:], p_idxu[:])
            idxv_sb = cst.tile([P, tiles_total * 8], I16)
            nc.sync.dma_start(idxv_sb[:], p_idxv[:])
            col_sb = cst.tile([P, tiles_total], GDT)
            nc.sync.dma_start(col_sb[:], p_col[:])
            gcnt_sb = cst.tile([1, ncalls], mybir.dt.int32)
            nc.sync.dma_start(gcnt_sb[:], p_gcnt[:])

            if prime:
                Kmax = int(max(K_list))
                nb_u = UBUFS if split_ag else 4
                for tg, nb in (("ub", nb_u), ("vb", 6 if split_ag else 4)):
                    for _ in range(nb):
                        t = gat.tile([P, Kmax * D], GDT, tag=tg,
                                     bufs=nb)
                        nc.vector.memset(t[:], 0.0)

            GMAX = 1024
            ni_regs = {}
            qrr = [0, 0]

            def _reg(ni):
                if ni not in ni_regs:
                    ni_regs[ni] = nc.gpsimd.to_reg(ni)
                return ni_regs[ni]

            iota3 = iota_sb[:].rearrange("p (k j) -> p k j", k=1)

            cregs = None
            crr = [0]
            if rt_counts and split_ag:
                cregs = [nc.alloc_register(mybir.EngineType.Pool,
                                           name=f"gcnt_reg{i}")
                         for i in range(16)]

            emit = _emit_iteration_split if split_ag else _emit_iteration
            for _it in range(repeat_all):
                emit(
                    nc, cfg, K_list, toff, GDT, sh, st, repeat_a, repeat_b,
                    GMAX, _reg, qrr, iota3, dramp, pa, psA, gat, smp, psB,
                    flp, w1_sb, w2_sb, b1_sb, b2_sb, ones_sb, iota_sb,
                    embT_sb, idxu_sb, idxv_sb, col_sb, p_out, repeat_ag,
                    coff, gcnt_sb, cregs, crr)

    nc.compile()
    return nc


def _emit_iteration_split(nc, cfg, K_list, toff, GDT, sh, st, repeat_a,
                          repeat_b, GMAX, _reg, qrr, iota3, dramp, pa, psA,
                          gat, smp, psB, flp, w1_sb, w2_sb, b1_sb, b2_sb,
                          ones_sb, iota_sb, embT_sb, idxu_sb, idxv_sb,
                          col_sb, p_out, repeat_ag=1, coff=None, gcnt_sb=None,
                          cregs=None, crr=None):
    """Split-table pipeline: R1 and R2 are separate tensors/collectives so
    AG(R1) overlaps computing R2, and AG(R2) overlaps the u-gathers (which
    only need full1).  u-gathers use SWDGE queues 0/1, v-gathers 2/3, so
    stalled v-gathers don't head-of-line-block u-gathers."""
    shard1 = dramp.tile([sh, D], GDT, name="shard1")
    shard2 = dramp.tile([sh, D], GDT, name="shard2")
    fulls = [None, None]

    # ---- Phase A + per-table AG ----
    for fi, (w_sb, b_sb, shard, fname) in enumerate(
            ((w1_sb, b1_sb, shard1, "full1"), (w2_sb, b2_sb, shard2,
                                               "full2"))):
        for j in [j for _ in range(repeat_a) for j in range(st)]:
            ps = psA.tile([P, D], F32)
            emb_j = embT_sb[:, j * P:(j + 1) * P]
            nc.tensor.matmul(ps[:], lhsT=emb_j, rhs=w_sb[:],
                             start=True, stop=False)
            nc.tensor.matmul(ps[:], lhsT=ones_sb[:], rhs=b_sb[:],
                             start=False, stop=True)
            rt = pa.tile([P, D], GDT)
            nc.scalar.activation(rt[:], ps[:],
                                 mybir.ActivationFunctionType.Relu)
            nc.sync.dma_start(shard[j * P:(j + 1) * P, :], rt[:])
        for _ag in range(repeat_ag):
            # Shared tensors allow one writer inst; fresh tile per AG
            full = dramp.tile([cfg.v_pad, D], GDT, addr_space="Shared",
                              name=fname)
            nc.gpsimd.collective_compute(
                "AllGather",
                mybir.AluOpType.bypass,
                replica_groups=[list(range(C))],
                ins=[shard.opt()],
                outs=[full.opt()],
            )
            fulls[fi] = full
    full1, full2 = fulls

    # ---- Phase B ----
    # u-gathers (needing only full1) run ahead of the v-gathers by RA
    # blocks, so the u stream keeps all 4 SWDGE queues busy while AG(R2)
    # is still in flight.  All calls round-robin the 4 queues.

    def _chunks(K, ni):
        nch = -(-ni // GMAX)
        csz = (-(-K // nch)) * P
        return [(c0, min(csz, ni - c0)) for c0 in range(0, ni, csz)]

    def emit_gather(b, buf, full, idx_sb):
        K = int(K_list[b])
        o8 = int(toff[b]) * 8
        for i, (c0, nc_) in enumerate(_chunks(K, K * P)):
            t0 = c0 // P
            nt = nc_ // P
            q = qrr[0] % 4
            qrr[0] += 1
            if cregs is not None:
                # runtime per-core true count: pad slots beyond it are
                # skipped (no descriptors); stale lanes are primed finite
                nreg = cregs[crr[0] % len(cregs)]
                crr[0] += 1
                e = coff[b] + i
                nc.gpsimd.reg_load(nreg, gcnt_sb[0:1, e:e + 1])
            else:
                nreg = _reg(nc_)
            nc.gpsimd.dma_gather(
                out_ap=buf[:, t0 * D:(t0 + nt) * D].rearrange(
                    "p (k d) -> p k d", d=D),
                in_ap=full[:],
                idxs_ap=idx_sb[:, o8 + c0 // 16:
                               o8 + c0 // 16 + nc_ // 16],
                num_idxs=nc_,
                num_idxs_reg=nreg,
                elem_size=D,
                queue_num=q,
            )

    for _rb in range(repeat_b):
        ubs = {}
        for b in range(min(RA, cfg.blocks_pc)):
            K = int(K_list[b])
            ubs[b] = gat.tile([P, K * D], GDT, tag="ub", bufs=UBUFS,
                               name="ub")
            emit_gather(b, ubs[b], full1, idxu_sb)
        for b in range(cfg.blocks_pc):
            K = int(K_list[b])
            ni = K * P
            ub = ubs.pop(b)
            vb = gat.tile([P, K * D], GDT, tag="vb", bufs=6)
            emit_gather(b, vb, full2, idxv_sb)
            ba = b + RA
            if ba < cfg.blocks_pc:
                Ka = int(K_list[ba])
                ubs[ba] = gat.tile([P, Ka * D], GDT, tag="ub", bufs=UBUFS,
                                    name="ub")
                emit_gather(ba, ubs[ba], full1, idxu_sb)
            ps = psB.tile([P, D], F32)
            g0 = int(toff[b])
            Sw = smp.tile([P, K * P], GDT, tag="S")
            nc.vector.tensor_tensor(
                out=Sw[:].rearrange("p (k j) -> p k j", j=P),
                in0=col_sb[:, g0:g0 + K].to_broadcast([P, K, P]),
                in1=iota3.to_broadcast([P, K, P]),
                op=mybir.AluOpType.is_equal,
            )
            mw = smp.tile([P, K * D], GDT, tag="m")
            for c0, nc_ in _chunks(K, ni):
                lo2, hi2 = (c0 // P) * D, (c0 // P + nc_ // P) * D
                nc.vector.tensor_tensor(
                    out=mw[:, lo2:hi2], in0=ub[:, lo2:hi2],
                    in1=vb[:, lo2:hi2], op=mybir.AluOpType.mult,
                )
            for t in range(K):
                nc.tensor.matmul(ps[:], lhsT=Sw[:, t * P:(t + 1) * P],
                                 rhs=mw[:, t * D:(t + 1) * D],
                                 start=(t == 0), stop=(t == K - 1))
            fl = flp.tile([P, D], GDT)
            nc.scalar.activation(fl[:], ps[:],
                                 mybir.ActivationFunctionType.Copy)
            nc.sync.dma_start(p_out[b * P:(b + 1) * P, :], fl[:])


def _emit_iteration(nc, cfg, K_list, toff, GDT, sh, st, repeat_a, repeat_b,
                    GMAX, _reg, qrr, iota3, dramp, pa, psA, gat, smp, psB,
                    flp, w1_sb, w2_sb, b1_sb, b2_sb, ones_sb, iota_sb,
                    embT_sb, idxu_sb, idxv_sb, col_sb, p_out, repeat_ag=1,
                    coff=None, gcnt_sb=None, cregs=None, crr=None):
    with _noop_ctx():
            shard = dramp.tile([sh, 2 * D], GDT, name="shard")

            # ---- Phase A: transformed table shard (R1 | R2) ----
            for j in [j for _ in range(repeat_a) for j in range(st)]:
                ps = psA.tile([P, 2 * D], F32)
                emb_j = embT_sb[:, j * P:(j + 1) * P]
                nc.tensor.matmul(ps[:, 0:D], lhsT=emb_j, rhs=w1_sb[:],
                                 start=True, stop=False)
                nc.tensor.matmul(ps[:, 0:D], lhsT=ones_sb[:], rhs=b1_sb[:],
                                 start=False, stop=True)
                nc.tensor.matmul(ps[:, D:2 * D], lhsT=emb_j, rhs=w2_sb[:],
                                 start=True, stop=False)
                nc.tensor.matmul(ps[:, D:2 * D], lhsT=ones_sb[:], rhs=b2_sb[:],
                                 start=False, stop=True)
                rt = pa.tile([P, 2 * D], GDT)
                nc.scalar.activation(rt[:], ps[:],
                                     mybir.ActivationFunctionType.Relu)
                nc.sync.dma_start(shard[j * P:(j + 1) * P, :], rt[:])

            for _ag in range(repeat_ag):
                full = dramp.tile([cfg.v_pad, 2 * D], GDT,
                                  addr_space="Shared", name="full")
                nc.gpsimd.collective_compute(
                    "AllGather",
                    mybir.AluOpType.bypass,
                    replica_groups=[list(range(C))],
                    ins=[shard.opt()],
                    outs=[full.opt()],
                )

            # ---- Phase B: gather + one-hot matmul segment-sum ----
            # dma_gather is chunked at 1024 indices/call (64-descriptor
            # packet limit per 16-partition lane) and spread over the 4
            # SWDGE queues.
            for b in [b for _ in range(repeat_b)
                      for b in range(cfg.blocks_pc)]:
                K = int(K_list[b])
                ni = K * P
                o8 = int(toff[b]) * 8
                ub = gat.tile([P, K * D], GDT, tag="ub")
                vb = gat.tile([P, K * D], GDT, tag="vb", bufs=6)
                # balanced tile-aligned chunks (<=GMAX) pipeline the 4 SWDGE
                # queues measurably better than GMAX+remainder; issuing each
                # chunk's u/v PAIR together lets that chunk's multiply start
                # as early as possible
                nch = -(-ni // GMAX)
                csz = (-(-K // nch)) * P
                for c0 in range(0, ni, csz):
                    for (buf, src_lo, idx_sb) in ((ub, 0, idxu_sb),
                                                  (vb, D, idxv_sb)):
                        nc_ = min(csz, ni - c0)
                        t0 = c0 // P
                        nt = nc_ // P
                        q = qrr[0] % 4
                        qrr[0] += 1
                        nc.gpsimd.dma_gather(
                            out_ap=buf[:, t0 * D:(t0 + nt) * D].rearrange(
                                "p (k d) -> p k d", d=D),
                            in_ap=full[:, src_lo:src_lo + D],
                            idxs_ap=idx_sb[:, o8 + c0 // 16:
                                           o8 + c0 // 16 + nc_ // 16],
                            num_idxs=nc_,
                            num_idxs_reg=_reg(nc_),
                            elem_size=D,
                            elem_step=2 * D,
                            queue_num=q,
                        )
                ps = psB.tile([P, D], F32)
                g0 = int(toff[b])
                Sw = smp.tile([P, K * P], GDT, tag="S")
                nc.vector.tensor_tensor(
                    out=Sw[:].rearrange("p (k j) -> p k j", j=P),
                    in0=col_sb[:, g0:g0 + K].to_broadcast([P, K, P]),
                    in1=iota3.to_broadcast([P, K, P]),
                    op=mybir.AluOpType.is_equal,
                )
                # multiply per gather-chunk so chunk-0 compute overlaps
                # chunk-1's gather tail
                mw = smp.tile([P, K * D], GDT, tag="m")
                for c0 in range(0, ni, csz):
                    nc_ = min(csz, ni - c0)
                    lo2, hi2 = (c0 // P) * D, (c0 // P + nc_ // P) * D
                    nc.vector.tensor_tensor(
                        out=mw[:, lo2:hi2], in0=ub[:, lo2:hi2],
                        in1=vb[:, lo2:hi2], op=mybir.AluOpType.mult,
                    )
                for t in range(K):
                    nc.tensor.matmul(ps[:], lhsT=Sw[:, t * P:(t + 1) * P],
                                     rhs=mw[:, t * D:(t + 1) * D],
                                     start=(t == 0), stop=(t == K - 1))
                fl = flp.tile([P, D], GDT)
                nc.scalar.activation(fl[:], ps[:],
                                     mybir.ActivationFunctionType.Copy)
                nc.sync.dma_start(p_out[b * P:(b + 1) * P, :], fl[:])


_nc_cache = {}


def kernel(emb_table, w1, bias1, w2, bias2, node_tokens, edge_tokens, src,
           dst):
    cfg = FULL_CFG
    in_maps, K_list, perms = host_prep(cfg, emb_table, w1, bias1, w2,
                                       bias2, node_tokens, edge_tokens, src,
                                       dst)
    key = tuple(int(k) for k in K_list)
    if key not in _nc_cache:
        _nc_cache[key] = build_nc(cfg, K_list)
    res = run_bass_kernel_spmd(_nc_cache[key], in_maps,
                               core_ids=list(range(C)))
    outs = []
    for c in range(C):
        oc = np.asarray(res.results[c]["out"]).reshape(cfg.blocks_pc, P, D)
        inv = np.empty_like(oc)
        inv[perms[c]] = oc              # slot j holds block perms[c, j]
        outs.append(inv.reshape(cfg.npc, D))
    out = np.concatenate(outs, axis=0)
    return np.ascontiguousarray(out[:cfg.n_nodes]).astype(np.float32)



# revision 57
# speedup vs baseline: 1.2701x; 1.2701x over previous
"""Trainium2 Bass kernel for GNN message-passing encoder.

Computes (reference semantics):
    node_h = relu(emb[node_tokens] @ w1 + b1)        [N, D]
    edge_h = relu(emb[edge_tokens] @ w2 + b2)        [E, D]
    msg    = node_h[src] * edge_h                    [E, D]
    out    = segment_sum(msg, dst, N)                [N, D]

Strategy (8 NeuronCores):
  * Algebraic rewrite: both MLPs act on embedding rows, so precompute
    transformed tables R1 = relu(emb@w1+b1), R2 = relu(emb@w2+b2)
    (VOCAB rows each) once, then the per-edge work is two row gathers
    (R1[node_tokens[src]], R2[edge_tokens]), an elementwise multiply and
    a segment-sum.  This removes the per-edge matmuls entirely.
  * Phase A (bf16 in, f16 tables out): each core computes a 1/8
    row-shard of R1 and R2 as SEPARATE tensors; two AllGathers replicate
    them.  AG(R1) overlaps computing R2; AG(R2) overlaps the u-gathers,
    which only need R1 and run RA=24 blocks ahead of the v-gathers
    (split pipeline, all 4 SWDGE queues round-robin).
  * Phase B: edges are sorted by dst and dst-range sharded across cores
    (6272 nodes = 49 slots of 128 per core).  Slots are rebalanced by
    order statistics (slot j serves each core's j-th largest block;
    kernel() un-permutes the output) to shrink SPMD max-over-core
    padding.  Per slot the edges are gathered with dma_gather (int16 row
    indices, 256B f16 rows), msg = u*v on DVE, and a one-hot matrix S
    (built on DVE from the per-edge local dst id vs an iota row) is used
    on the PE: PSUM[128 nodes, D] += S^T @ msg accumulated over the
    slot's edge tiles -- a dense segment-sum with no scatter and no
    extra collective.
"""

import contextlib

import numpy as np

import ml_dtypes

import concourse.bacc as bacc
import concourse.bass as bass
import concourse.mybir as mybir
import concourse.tile as tile
from concourse import library_config
from concourse.bass_utils import run_bass_kernel_spmd

F32 = mybir.dt.float32
F16 = mybir.dt.float16
BF16 = mybir.dt.bfloat16
I16 = mybir.dt.int16
NP_BF16 = ml_dtypes.bfloat16
_noop_ctx = contextlib.nullcontext

C = 8          # cores
D = 128        # feature dim
P = 128        # partitions
RA = 24        # u-gather runahead blocks (split pipeline)
UBUFS = RA + 2


class Cfg:
    def __init__(self, n_nodes, n_edges, vocab, v_pad, blocks_pc):
        self.n_nodes = n_nodes
        self.n_edges = n_edges
        self.vocab = vocab
        self.v_pad = v_pad              # multiple of 8*128
        self.blocks_pc = blocks_pc      # node blocks (128 nodes) per core
        self.npc = blocks_pc * P        # nodes per core (padded)
        assert self.npc * C >= n_nodes
        assert v_pad % (C * P) == 0
        assert v_pad <= 32768           # int16 gather indices


FULL_CFG = Cfg(n_nodes=50000, n_edges=600000, vocab=32000, v_pad=32768,
               blocks_pc=49)


def host_prep(cfg, emb_table, w1, bias1, w2, bias2, node_tokens, edge_tokens,
              src, dst, neg_pads=False):
    """Pure index/layout prep on host -> per-core input maps + K_list."""
    emb_table = np.asarray(emb_table, np.float32)
    w1 = np.asarray(w1, NP_BF16)
    w2 = np.asarray(w2, NP_BF16)
    bias1 = np.asarray(bias1, NP_BF16).reshape(1, D)
    bias2 = np.asarray(bias2, NP_BF16).reshape(1, D)
    node_tokens = np.asarray(node_tokens).astype(np.int64)
    edge_tokens = np.asarray(edge_tokens).astype(np.int64)
    src = np.asarray(src).astype(np.int64)
    dst = np.asarray(dst).astype(np.int64)

    stok = node_tokens[src]                      # token feeding node_h per edge
    order = np.argsort(dst, kind="stable")
    dstS = dst[order]
    stokS = stok[order]
    etokS = edge_tokens[order]
    gdt_np = np.float16

    nblk = C * cfg.blocks_pc
    bounds = np.searchsorted(dstS, np.arange(nblk + 1) * P)
    cnt = np.diff(bounds).reshape(C, cfg.blocks_pc)   # [C, blocks_pc]
    # order-statistic rebalancing: slot j holds each core's j-th largest
    # block, so the SPMD max-over-cores padding shrinks from ~8% to ~2%.
    # perms[c, j] = original block index served in slot j on core c; the
    # caller un-permutes the per-slot output rows.
    perms = np.argsort(-cnt, axis=1, kind="stable")   # [C, blocks_pc]
    cnt_slot = np.take_along_axis(cnt, perms, axis=1)
    nmax = cnt_slot.max(axis=0)                       # per slot max count
    K_list = np.maximum(1, -(-nmax // P)).astype(int)  # tiles per slot
    toff = np.concatenate([[0], np.cumsum(K_list)]).astype(int)
    tiles_total = int(toff[-1])
    slots = tiles_total * P

    sh = cfg.v_pad // C

    iota = np.tile(np.arange(P, dtype=gdt_np)[None, :], (P, 1))
    ones = np.ones((1, D), NP_BF16)

    def pack16(a):
        # gather index packing: idx k lives at [k%16, k//16], replicated to
        # 128 partitions (8 gpsimd cores x 16)
        return np.ascontiguousarray(np.tile(a.reshape(-1, 16).T, (8, 1)))

    # pad slots gather real rows spread over the table (the one-hot S has
    # zero rows there, killing their contribution).  NOTE: trailing -1
    # indices (neg_pads=True), although documented as "ignored", wedge the
    # device on this runtime even with >=1 valid index per call -- do not
    # re-enable without re-testing.
    spread = (np.arange(slots, dtype=np.int64) * 97 % cfg.vocab).astype(
        np.int16)
    pad_fill = np.full(slots, -1, np.int16) if neg_pads else spread
    in_maps = []
    for c in range(C):
        u16 = pad_fill.copy()
        v16 = pad_fill.copy()
        col = np.full(slots, -1.0, gdt_np)
        for j in range(cfg.blocks_pc):
            b = int(perms[c, j])
            gb = c * cfg.blocks_pc + b
            s0, s1 = bounds[gb], bounds[gb + 1]
            m = s1 - s0
            o = toff[j] * P
            u16[o:o + m] = stokS[s0:s1].astype(np.int16)
            v16[o:o + m] = etokS[s0:s1].astype(np.int16)
            col[o:o + m] = (dstS[s0:s1] - (c * cfg.npc + b * P)).astype(
                gdt_np)

        if neg_pads:
            # a fully-negative gather call (0 descriptors) wedges the
            # device; keep >=1 valid index at each call's start
            for b in range(cfg.blocks_pc):
                K = int(K_list[b])
                ni = K * P
                m = int(cnt[c, b])
                nch = -(-ni // 1024)            # GMAX
                csz = (-(-K // nch)) * P
                o = toff[b] * P
                for c0 in range(0, ni, csz):
                    if c0 >= m:
                        u16[o + c0] = 0
                        v16[o + c0] = 0

        # per-gather-call true edge counts (shared by the u and v call of
        # the same chunk): clamp to >=16 so no call is empty, and to the
        # chunk size.  Slots beyond the count keep stale-but-finite data
        # (primed buffers) and are killed by the zero rows of S.
        gcnt = []
        for j in range(cfg.blocks_pc):
            K = int(K_list[j])
            ni = K * P
            m = int(cnt_slot[c, j])
            nch = -(-ni // 1024)                # GMAX
            csz = (-(-K // nch)) * P
            for c0 in range(0, ni, csz):
                gcnt.append(int(np.clip(m - c0, 16, min(csz, ni - c0))))
        gcnt = np.asarray(gcnt, np.int32).reshape(1, -1)

        shard = np.zeros((sh, D), np.float32)
        lo, hi = c * sh, min((c + 1) * sh, cfg.vocab)
        if lo < cfg.vocab:
            shard[:hi - lo] = emb_table[lo:hi]

        in_maps.append({
            "emb_shT": np.ascontiguousarray(shard.T).astype(
                NP_BF16),                                   # [128, sh]
            "w1": w1, "w2": w2, "b1": bias1, "b2": bias2,
            "ones": ones, "iota": iota,
            "idx_u": pack16(u16),                           # [128, tiles*8]
            "idx_v": pack16(v16),
            "gcnt": gcnt,                                   # [1, ncalls]
            "col": np.ascontiguousarray(
                col.reshape(tiles_total, P).T),             # [128, tiles]
        })
    return in_maps, K_list, perms


def _call_offsets(cfg, K_list):
    """Per-block offsets into the per-call gcnt array (chunk structure must
    match _chunks)."""
    coff = []
    n = 0
    for b in range(cfg.blocks_pc):
        coff.append(n)
        K = int(K_list[b])
        ni = K * P
        nch = -(-ni // 1024)
        csz = (-(-K // nch)) * P
        n += len(range(0, ni, csz))
    return coff, n


def build_nc(cfg, K_list, repeat_b=1, repeat_a=1, repeat_all=1,
             table_dtype=F16, prime=True, repeat_ag=1, split_ag=True,
             rt_counts=False):
    # NOTE rt_counts (runtime per-call gather counts via gpsimd reg_load)
    # hangs the device: the tile scheduler does not order reg_load vs the
    # consuming dma_gather, so counts race.  Left off.
    """repeat_* repeat phase bodies / the whole pipeline inside one NEFF;
    only used by the timing harness to amortize launch overhead (results
    unchanged)."""
    GDT = table_dtype
    sh = cfg.v_pad // C
    st = sh // P                      # shard tiles (phase A)
    toff = np.concatenate([[0], np.cumsum(K_list)]).astype(int)
    tiles_total = int(toff[-1])
    coff, ncalls = _call_offsets(cfg, K_list)

    nc = bacc.Bacc("TRN2", target_bir_lowering=False, num_devices=C,
                   num_swdge_queues=4)

    p_embT = nc.declare_dram_parameter("emb_shT", [P, sh], BF16,
                                       isOutput=False)
    p_w1 = nc.declare_dram_parameter("w1", [D, D], BF16, isOutput=False)
    p_w2 = nc.declare_dram_parameter("w2", [D, D], BF16, isOutput=False)
    p_b1 = nc.declare_dram_parameter("b1", [1, D], BF16, isOutput=False)
    p_b2 = nc.declare_dram_parameter("b2", [1, D], BF16, isOutput=False)
    p_ones = nc.declare_dram_parameter("ones", [1, D], BF16, isOutput=False)
    p_iota = nc.declare_dram_parameter("iota", [P, P], GDT, isOutput=False)
    p_idxu = nc.declare_dram_parameter("idx_u", [P, tiles_total * 8], I16,
                                       isOutput=False)
    p_idxv = nc.declare_dram_parameter("idx_v", [P, tiles_total * 8], I16,
                                       isOutput=False)
    p_col = nc.declare_dram_parameter("col", [P, tiles_total], GDT,
                                      isOutput=False)
    p_gcnt = nc.declare_dram_parameter("gcnt", [1, ncalls], mybir.dt.int32,
                                       isOutput=False)
    p_out = nc.declare_dram_parameter("out", [cfg.npc, D], GDT,
                                      isOutput=True)

    with tile.TileContext(nc) as tc:
        with (
            tc.tile_pool(name="dram", bufs=1, space="DRAM") as dramp,
            tc.tile_pool(name="cst", bufs=1) as cst,
            tc.tile_pool(name="pa", bufs=3) as pa,
            tc.tile_pool(name="psA", bufs=2, space="PSUM") as psA,
            tc.tile_pool(name="gat", bufs=4) as gat,
            tc.tile_pool(name="sm", bufs=6) as smp,
            tc.tile_pool(name="psB", bufs=4, space="PSUM") as psB,
            tc.tile_pool(name="fl", bufs=3) as flp,
        ):
            w1_sb = cst.tile([D, D], BF16)
            nc.sync.dma_start(w1_sb[:], p_w1[:])
            w2_sb = cst.tile([D, D], BF16)
            nc.sync.dma_start(w2_sb[:], p_w2[:])
            b1_sb = cst.tile([1, D], BF16)
            nc.sync.dma_start(b1_sb[:], p_b1[:])
            b2_sb = cst.tile([1, D], BF16)
            nc.sync.dma_start(b2_sb[:], p_b2[:])
            ones_sb = cst.tile([1, D], BF16)
            nc.sync.dma_start(ones_sb[:], p_ones[:])
            iota_sb = cst.tile([P, P], GDT)
            nc.sync.dma_start(iota_sb[:], p_iota[:])
            embT_sb = cst.tile([P, sh], BF16)
            nc.sync.dma_start(embT_sb[:], p_embT[:])
            idxu_sb = cst.tile([P, tiles_total * 8], I16)
            nc.sync.dma_start(idxu_sb[:], p_idxu[:])
            idxv_sb = cst.tile([P, tiles_total * 8], I16)
            nc.sync.dma_start(idxv_sb[:], p_idxv[:])
            col_sb = cst.tile([P, tiles_total], GDT)
            nc.sync.dma_start(col_sb[:], p_col[:])
            gcnt_sb = cst.tile([1, ncalls], mybir.dt.int32)
            nc.sync.dma_start(gcnt_sb[:], p_gcnt[:])

            if prime:
                Kmax = int(max(K_list))
                nb_u = UBUFS if split_ag else 4
                for tg, nb in (("ub", nb_u), ("vb", 6 if split_ag else 4)):
                    for _ in range(nb):
                        t = gat.tile([P, Kmax * D], GDT, tag=tg,
                                     bufs=nb)
                        nc.vector.memset(t[:], 0.0)

            GMAX = 1024
            ni_regs = {}
            qrr = [0, 0]

            def _reg(ni):
                if ni not in ni_regs:
                    ni_regs[ni] = nc.gpsimd.to_reg(ni)
                return ni_regs[ni]

            iota3 = iota_sb[:].rearrange("p (k j) -> p k j", k=1)

            cregs = None
            crr = [0]
            if rt_counts and split_ag:
                cregs = [nc.alloc_register(mybir.EngineType.Pool,
                                           name=f"gcnt_reg{i}")
                         for i in range(16)]

            emit = _emit_iteration_split if split_ag else _emit_iteration
            for _it in range(repeat_all):
                emit(
                    nc, cfg, K_list, toff, GDT, sh, st, repeat_a, repeat_b,
                    GMAX, _reg, qrr, iota3, dramp, pa, psA, gat, smp, psB,
                    flp, w1_sb, w2_sb, b1_sb, b2_sb, ones_sb, iota_sb,
                    embT_sb, idxu_sb, idxv_sb, col_sb, p_out, repeat_ag,
                    coff, gcnt_sb, cregs, crr)

    nc.compile()
    return nc


def _emit_iteration_split(nc, cfg, K_list, toff, GDT, sh, st, repeat_a,
                          repeat_b, GMAX, _reg, qrr, iota3, dramp, pa, psA,
                          gat, smp, psB, flp, w1_sb, w2_sb, b1_sb, b2_sb,
                          ones_sb, iota_sb, embT_sb, idxu_sb, idxv_sb,
                          col_sb, p_out, repeat_ag=1, coff=None, gcnt_sb=None,
                          cregs=None, crr=None):
    """Split-table pipeline: R1 and R2 are separate tensors/collectives so
    AG(R1) overlaps computing R2, and AG(R2) overlaps the u-gathers (which
    only need full1).  u-gathers use SWDGE queues 0/1, v-gathers 2/3, so
    stalled v-gathers don't head-of-line-block u-gathers."""
    shard1 = dramp.tile([sh, D], GDT, name="shard1")
    shard2 = dramp.tile([sh, D], GDT, name="shard2")
    fulls = [None, None]

    # ---- Phase A + per-table AG ----
    for fi, (w_sb, b_sb, shard, fname) in enumerate(
            ((w1_sb, b1_sb, shard1, "full1"), (w2_sb, b2_sb, shard2,
                                               "full2"))):
        for j in [j for _ in range(repeat_a) for j in range(st)]:
            ps = psA.tile([P, D], F32)
            emb_j = embT_sb[:, j * P:(j + 1) * P]
            nc.tensor.matmul(ps[:], lhsT=emb_j, rhs=w_sb[:],
                             start=True, stop=False)
            nc.tensor.matmul(ps[:], lhsT=ones_sb[:], rhs=b_sb[:],
                             start=False, stop=True)
            rt = pa.tile([P, D], GDT)
            nc.scalar.activation(rt[:], ps[:],
                                 mybir.ActivationFunctionType.Relu)
            nc.sync.dma_start(shard[j * P:(j + 1) * P, :], rt[:])
        for _ag in range(repeat_ag):
            # Shared tensors allow one writer inst; fresh tile per AG
            full = dramp.tile([cfg.v_pad, D], GDT, addr_space="Shared",
                              name=fname)
            nc.gpsimd.collective_compute(
                "AllGather",
                mybir.AluOpType.bypass,
                replica_groups=[list(range(C))],
                ins=[shard.opt()],
                outs=[full.opt()],
            )
            fulls[fi] = full
    full1, full2 = fulls

    # ---- Phase B ----
    # u-gathers (needing only full1) run ahead of the v-gathers by RA
    # blocks, so the u stream keeps all 4 SWDGE queues busy while AG(R2)
    # is still in flight.  All calls round-robin the 4 queues.

    def _chunks(K, ni):
        nch = -(-ni // GMAX)
        csz = (-(-K // nch)) * P
        return [(c0, min(csz, ni - c0)) for c0 in range(0, ni, csz)]

    def emit_gather(b, buf, full, idx_sb):
        K = int(K_list[b])
        o8 = int(toff[b]) * 8
        for i, (c0, nc_) in enumerate(_chunks(K, K * P)):
            t0 = c0 // P
            nt = nc_ // P
            q = qrr[0] % 4
            qrr[0] += 1
            if cregs is not None:
                # runtime per-core true count: pad slots beyond it are
                # skipped (no descriptors); stale lanes are primed finite
                nreg = cregs[crr[0] % len(cregs)]
                crr[0] += 1
                e = coff[b] + i
                nc.gpsimd.reg_load(nreg, gcnt_sb[0:1, e:e + 1])
            else:
                nreg = _reg(nc_)
            nc.gpsimd.dma_gather(
                out_ap=buf[:, t0 * D:(t0 + nt) * D].rearrange(
                    "p (k d) -> p k d", d=D),
                in_ap=full[:],
                idxs_ap=idx_sb[:, o8 + c0 // 16:
                               o8 + c0 // 16 + nc_ // 16],
                num_idxs=nc_,
                num_idxs_reg=nreg,
                elem_size=D,
                queue_num=q,
            )

    for _rb in range(repeat_b):
        ubs = {}
        for b in range(min(RA, cfg.blocks_pc)):
            K = int(K_list[b])
            ubs[b] = gat.tile([P, K * D], GDT, tag="ub", bufs=UBUFS,
                               name="ub")
            emit_gather(b, ubs[b], full1, idxu_sb)
        for b in range(cfg.blocks_pc):
            K = int(K_list[b])
            ni = K * P
            ub = ubs.pop(b)
            vb = gat.tile([P, K * D], GDT, tag="vb", bufs=6)
            emit_gather(b, vb, full2, idxv_sb)
            ba = b + RA
            if ba < cfg.blocks_pc:
                Ka = int(K_list[ba])
                ubs[ba] = gat.tile([P, Ka * D], GDT, tag="ub", bufs=UBUFS,
                                    name="ub")
                emit_gather(ba, ubs[ba], full1, idxu_sb)
            ps = psB.tile([P, D], F32)
            g0 = int(toff[b])
            Sw = smp.tile([P, K * P], GDT, tag="S")
            nc.vector.tensor_tensor(
                out=Sw[:].rearrange("p (k j) -> p k j", j=P),
                in0=col_sb[:, g0:g0 + K].to_broadcast([P, K, P]),
                in1=iota3.to_broadcast([P, K, P]),
                op=mybir.AluOpType.is_equal,
            )
            mw = smp.tile([P, K * D], GDT, tag="m")
            for c0, nc_ in _chunks(K, ni):
                lo2, hi2 = (c0 // P) * D, (c0 // P + nc_ // P) * D
                nc.vector.tensor_tensor(
                    out=mw[:, lo2:hi2], in0=ub[:, lo2:hi2],
                    in1=vb[:, lo2:hi2], op=mybir.AluOpType.mult,
                )
            for t in range(K):
                nc.tensor.matmul(ps[:], lhsT=Sw[:, t * P:(t + 1) * P],
                                 rhs=mw[:, t * D:(t + 1) * D],
                                 start=(t == 0), stop=(t == K - 1))
            fl = flp.tile([P, D], GDT)
            nc.scalar.activation(fl[:], ps[:],
                                 mybir.ActivationFunctionType.Copy)
            nc.sync.dma_start(p_out[b * P:(b + 1) * P, :], fl[:])


def _emit_iteration(nc, cfg, K_list, toff, GDT, sh, st, repeat_a, repeat_b,
                    GMAX, _reg, qrr, iota3, dramp, pa, psA, gat, smp, psB,
                    flp, w1_sb, w2_sb, b1_sb, b2_sb, ones_sb, iota_sb,
                    embT_sb, idxu_sb, idxv_sb, col_sb, p_out, repeat_ag=1,
                    coff=None, gcnt_sb=None, cregs=None, crr=None):
    with _noop_ctx():
            shard = dramp.tile([sh, 2 * D], GDT, name="shard")

            # ---- Phase A: transformed table shard (R1 | R2) ----
            for j in [j for _ in range(repeat_a) for j in range(st)]:
                ps = psA.tile([P, 2 * D], F32)
                emb_j = embT_sb[:, j * P:(j + 1) * P]
                nc.tensor.matmul(ps[:, 0:D], lhsT=emb_j, rhs=w1_sb[:],
                                 start=True, stop=False)
                nc.tensor.matmul(ps[:, 0:D], lhsT=ones_sb[:], rhs=b1_sb[:],
                                 start=False, stop=True)
                nc.tensor.matmul(ps[:, D:2 * D], lhsT=emb_j, rhs=w2_sb[:],
                                 start=True, stop=False)
                nc.tensor.matmul(ps[:, D:2 * D], lhsT=ones_sb[:], rhs=b2_sb[:],
                                 start=False, stop=True)
                rt = pa.tile([P, 2 * D], GDT)
                nc.scalar.activation(rt[:], ps[:],
                                     mybir.ActivationFunctionType.Relu)
                nc.sync.dma_start(shard[j * P:(j + 1) * P, :], rt[:])

            for _ag in range(repeat_ag):
                full = dramp.tile([cfg.v_pad, 2 * D], GDT,
                                  addr_space="Shared", name="full")
                nc.gpsimd.collective_compute(
                    "AllGather",
                    mybir.AluOpType.bypass,
                    replica_groups=[list(range(C))],
                    ins=[shard.opt()],
                    outs=[full.opt()],
                )

            # ---- Phase B: gather + one-hot matmul segment-sum ----
            # dma_gather is chunked at 1024 indices/call (64-descriptor
            # packet limit per 16-partition lane) and spread over the 4
            # SWDGE queues.
            for b in [b for _ in range(repeat_b)
                      for b in range(cfg.blocks_pc)]:
                K = int(K_list[b])
                ni = K * P
                o8 = int(toff[b]) * 8
                ub = gat.tile([P, K * D], GDT, tag="ub")
                vb = gat.tile([P, K * D], GDT, tag="vb", bufs=6)
                # balanced tile-aligned chunks (<=GMAX) pipeline the 4 SWDGE
                # queues measurably better than GMAX+remainder; issuing each
                # chunk's u/v PAIR together lets that chunk's multiply start
                # as early as possible
                nch = -(-ni // GMAX)
                csz = (-(-K // nch)) * P
                for c0 in range(0, ni, csz):
                    for (buf, src_lo, idx_sb) in ((ub, 0, idxu_sb),
                                                  (vb, D, idxv_sb)):
                        nc_ = min(csz, ni - c0)
                        t0 = c0 // P
                        nt = nc_ // P
                        q = qrr[0] % 4
                        qrr[0] += 1
                        nc.gpsimd.dma_gather(
                            out_ap=buf[:, t0 * D:(t0 + nt) * D].rearrange(
                                "p (k d) -> p k d", d=D),
                            in_ap=full[:, src_lo:src_lo + D],
                            idxs_ap=idx_sb[:, o8 + c0 // 16:
                                           o8 + c0 // 16 + nc_ // 16],
                            num_idxs=nc_,
                            num_idxs_reg=_reg(nc_),
                            elem_size=D,
                            elem_step=2 * D,
                            queue_num=q,
                        )
                ps = psB.tile([P, D], F32)
                g0 = int(toff[b])
                Sw = smp.tile([P, K * P], GDT, tag="S")
                nc.vector.tensor_tensor(
                    out=Sw[:].rearrange("p (k j) -> p k j", j=P),
                    in0=col_sb[:, g0:g0 + K].to_broadcast([P, K, P]),
                    in1=iota3.to_broadcast([P, K, P]),
                    op=mybir.AluOpType.is_equal,
                )
                # multiply per gather-chunk so chunk-0 compute overlaps
                # chunk-1's gather tail
                mw = smp.tile([P, K * D], GDT, tag="m")
                for c0 in range(0, ni, csz):
                    nc_ = min(csz, ni - c0)
                    lo2, hi2 = (c0 // P) * D, (c0 // P + nc_ // P) * D
                    nc.vector.tensor_tensor(
                        out=mw[:, lo2:hi2], in0=ub[:, lo2:hi2],
                        in1=vb[:, lo2:hi2], op=mybir.AluOpType.mult,
                    )
                for t in range(K):
                    nc.tensor.matmul(ps[:], lhsT=Sw[:, t * P:(t + 1) * P],
                                     rhs=mw[:, t * D:(t + 1) * D],
                                     start=(t == 0), stop=(t == K - 1))
                fl = flp.tile([P, D], GDT)
                nc.scalar.activation(fl[:], ps[:],
                                     mybir.ActivationFunctionType.Copy)
                nc.sync.dma_start(p_out[b * P:(b + 1) * P, :], fl[:])


_nc_cache = {}


def kernel(emb_table, w1, bias1, w2, bias2, node_tokens, edge_tokens, src,
           dst):
    cfg = FULL_CFG
    in_maps, K_list, perms = host_prep(cfg, emb_table, w1, bias1, w2,
                                       bias2, node_tokens, edge_tokens, src,
                                       dst)
    key = tuple(int(k) for k in K_list)
    if key not in _nc_cache:
        _nc_cache[key] = build_nc(cfg, K_list)
    res = run_bass_kernel_spmd(_nc_cache[key], in_maps,
                               core_ids=list(range(C)))
    outs = []
    for c in range(C):
        oc = np.asarray(res.results[c]["out"]).reshape(cfg.blocks_pc, P, D)
        inv = np.empty_like(oc)
        inv[perms[c]] = oc              # slot j holds block perms[c, j]
        outs.append(inv.reshape(cfg.npc, D))
    out = np.concatenate(outs, axis=0)
    return np.ascontiguousarray(out[:cfg.n_nodes]).astype(np.float32)

